# revision 23
# baseline (speedup 1.0000x reference)
"""Trainium2 Bass kernel for nn_EvalCriterion (segment_reduce confusion counts).

Problem: windows of length W=30 are overlap-added onto a [B, L] grid
(L = S + W - 1), averaged by nonzero-contribution count, thresholded
(sigmoid(avg_i) > t for predictions, trunc(avg_t) for binary labels), and
reduced to four global confusion counts (TP, TN, FP, FN).

Math used by this kernel (valid for the graded configuration):
  * lens_index == arange(N).reshape(B, S)  -> the gather is a plain reshape.
  * t == 0.5  -> sigmoid(acc/cnt) > 0.5  <=>  acc > 0 (cnt is always >= 1),
    so the nonzero-count divisor never needs to be computed.
  * target values are in {0, 1} -> trunc(acc_t/cnt_t) == (acc_t > 0).
  So only the overlap-add sums acc_i, acc_t are needed, then sign tests and
  three global counts: CI = sum(raw_i), CT = sum(raw_t), TP = sum(raw_i*raw_t).
  FP = CI - TP, FN = CT - TP, TN = B*L - CI - CT + TP.

Sharding: data-parallel over B across 8 cores (2 batches per core). Each core
streams its 2x[S, W] blocks of input and target through SBUF once.

Per-core layout: for one batch, partition r holds rows [512r, 512r + 512).
The overlap-add over those rows is a single strided tensor_reduce: with
o' = W-1-o, position q's sum reads offsets 30q + 29o' (all-positive strides,
pairwise distinct), over a tile with 841-element zero guards on both sides.
Positions q in [0, 29) then still need the previous partition's tail
(q in [512, 541)), merged with one partition-shifted SBUF copy + add.
"""

import numpy as np

W = 30
B, S = 16, 65536
N = B * S
L = S + W - 1
NCORES = 8
BPC = B // NCORES          # batches per core = 2
SHARD_ROWS = BPC * S       # rows of input/target per core
P = 128                    # SBUF partitions
RPP = S // P               # rows per partition per batch = 512
LPART = RPP + W - 1        # local acc length = 541
DATA = RPP * W             # data elements per partition = 15360
GUARD = (W - 1) * (W - 1)  # zero guard on each side = 841
TOT = DATA + 2 * GUARD     # tile free size = 17042
K_DVE = 18                 # window offsets summed by the DVE strided reduce
GP_OFFS = W - K_DVE        # window offsets summed by GpSimd shifted adds

_NC = None
LAST_RESULTS = None        # BassKernelResults of the most recent device run


def _build_nc():
    import concourse.bacc as bacc
    import concourse.bass as bass
    import concourse.mybir as mybir
    from concourse.tile import TileContext

    f32 = mybir.dt.float32
    nc = bacc.Bacc(
        "TRN2",
        target_bir_lowering=False,
        debug=False,
        enable_asserts=False,
        num_devices=NCORES,
    )
    inp = nc.dram_tensor("inp", [SHARD_ROWS, W], f32, kind="ExternalInput").ap()
    tar = nc.dram_tensor("tar", [SHARD_ROWS, W], f32, kind="ExternalInput").ap()
    out = nc.dram_tensor("out", [P, 16], f32, kind="ExternalOutput").ap()

    with TileContext(nc) as tc:
        with (
            tc.tile_pool(name="main", bufs=1) as pool,
            tc.tile_pool(name="psum", bufs=1, space="PSUM") as psum,
        ):
            big = [pool.tile([P, TOT], f32, name=f"big{i}", tag=f"big{i}") for i in range(2)]
            parts = [pool.tile([P, LPART], f32, name=f"part{i}", tag=f"part{i}") for i in range(4)]
            accg = [pool.tile([P, LPART], f32, name=f"accg{i}", tag=f"accg{i}") for i in range(4)]
            planes = [pool.tile([P, RPP], f32, name=f"plane{i}", tag=f"plane{i}") for i in range(3)]
            cnts = pool.tile([P, 16], f32, name="cnts", tag="cnts")

            wsh = pool.tile([P, P], f32, name="wsh", tag="wsh")
            iden = pool.tile([P, P], f32, name="iden", tag="iden")
            ii = pool.tile([P, P], mybir.dt.int32, name="ii", tag="ii")
            # ii[pi, j] = j - pi; W[pi, po] = 1 iff po == pi - 1 (shift up),
            # iden = identity. Matmuls with these as lhsT copy/shift rows
            # into PSUM with accumulation.
            nc.gpsimd.iota(ii[:], pattern=[[1, P]], base=0, channel_multiplier=-1)
            nc.vector.tensor_scalar(
                out=wsh[:], in0=ii[:], scalar1=-1.0, scalar2=None,
                op0=mybir.AluOpType.is_equal,
            )
            nc.vector.tensor_scalar(
                out=iden[:], in0=ii[:], scalar1=0.0, scalar2=None,
                op0=mybir.AluOpType.is_equal,
            )
            nc.gpsimd.memset(cnts[:], 0.0)
            for t in big:
                nc.gpsimd.memset(t[:, 0:GUARD], 0.0)
                nc.gpsimd.memset(t[:, GUARD + DATA : TOT], 0.0)

            srcs = (inp, tar)
            pscs = []
            # Stream order: (b0, inp), (b0, tar), (b1, inp), (b1, tar);
            # ping-pong between the two big tiles for DMA/compute overlap.
            # Window offsets are split between engines: the DVE sums
            # o in [GP_OFFS, W) with one strided reduce, GpSimd sums
            # o in [0, GP_OFFS) with shifted adds into accg. The DVE combine
            # of accg into parts is deferred one tile so the DVE never
            # stalls waiting on GpSimd.
            for k in range(2 * BPC):
                b, which = divmod(k, 2)
                t = big[k % 2]
                view = srcs[which][b * S : (b + 1) * S, :].rearrange(
                    "(r m) w -> r (m w)", r=P
                )
                full = t[:]
                last = k == 2 * BPC - 1
                kdve = W if last else K_DVE
                NCH = 8 if k == 0 else 4
                CH = RPP // NCH
                qb = [CH * i for i in range(NCH)] + [LPART]
                for ci in range(NCH):
                    nc.sync.dma_start(
                        out=t[:, GUARD + ci * CH * W : GUARD + (ci + 1) * CH * W],
                        in_=view[:, ci * CH * W : (ci + 1) * CH * W],
                    )
                    q0, q1 = qb[ci], qb[ci + 1]
                    red = bass.AP(
                        tensor=full.tensor,
                        offset=full.offset + q0 * W,
                        ap=[list(full.ap[0]), [W, q1 - q0], [W - 1, kdve]],
                    )
                    nc.vector.tensor_reduce(
                        out=parts[k][:, q0:q1],
                        in_=red,
                        axis=mybir.AxisListType.X,
                        op=mybir.AluOpType.add,
                    )
                if last:
                    # all offsets on the DVE; merge boundary in SBUF below
                    pscs.append(None)
                    continue
                nc.gpsimd.memset(accg[k][:], 0.0)
                for o in range(GP_OFFS):
                    srcap = bass.AP(
                        tensor=full.tensor,
                        offset=full.offset + GUARD + o,
                        ap=[list(full.ap[0]), [W, RPP]],
                    )
                    nc.gpsimd.tensor_tensor(
                        out=accg[k][:, o : o + RPP],
                        in0=accg[k][:, o : o + RPP],
                        in1=srcap,
                        op=mybir.AluOpType.add,
                    )
                # PE: psc[k] = parts[k] + accg[k] with the boundary merge
                # (shifted heads) accumulated into the tail columns.
                pc = psum.tile([P, LPART], f32, name=f"psc{k}", tag=f"psc{k}")
                pscs.append(pc)
                nc.tensor.matmul(
                    pc[:, 0:RPP], iden[:], parts[k][:, 0:RPP],
                    start=True, stop=False,
                )
                nc.tensor.matmul(
                    pc[:, 0:RPP], iden[:], accg[k][:, 0:RPP],
                    start=False, stop=True,
                )
                nc.tensor.matmul(
                    pc[:, RPP:LPART], iden[:], parts[k][:, RPP:LPART],
                    start=True, stop=False,
                )
                nc.tensor.matmul(
                    pc[:, RPP:LPART], iden[:], accg[k][:, RPP:LPART],
                    start=False, stop=False,
                )
                nc.tensor.matmul(
                    pc[:, RPP:LPART], wsh[:], parts[k][:, 0 : W - 1],
                    start=False, stop=False,
                )
                nc.tensor.matmul(
                    pc[:, RPP:LPART], wsh[:], accg[k][:, 0 : W - 1],
                    start=False, stop=True,
                )

            # boundary merge for the last (all-DVE) tile, in SBUF
            kl = 2 * BPC - 1
            psl = psum.tile([P, W - 1], f32, name="psl", tag="psl")
            nc.tensor.matmul(
                psl[:], wsh[:], parts[kl][:, 0 : W - 1], start=True, stop=True
            )
            nc.any.tensor_tensor(
                out=parts[kl][:, RPP:LPART],
                in0=parts[kl][:, RPP:LPART],
                in1=psl[:],
                op=mybir.AluOpType.add,
            )

            for b in range(BPC):
                # Count region A: q in [29, 541) on all partitions (interior +
                # merged boundary), region B: partition 0's head q in [0, 29).
                # Every one of the batch's L positions is counted exactly once.
                # CT and raw_t go through the Scalar engine: acc_t >= 0, so
                # sign(acc_t) == (acc_t > 0) exactly.
                c0 = 6 * b
                for reg, cA in ((0, c0), (1, c0 + 3)):
                    if reg == 0:
                        sl = lambda pt: pt[:, W - 1 : LPART]
                        pl = lambda i: planes[i][:]
                        cn = lambda col: cnts[:, col : col + 1]
                    else:
                        sl = lambda pt: pt[0:1, 0 : W - 1]
                        pl = lambda i: planes[i][0:1, 0 : W - 1]
                        cn = lambda col: cnts[0:1, col : col + 1]
                    src_i = pscs[2 * b] if pscs[2 * b] is not None else parts[2 * b]
                    src_t = pscs[2 * b + 1] if pscs[2 * b + 1] is not None else parts[2 * b + 1]
                    nc.vector.tensor_scalar(
                        out=pl(0),
                        in0=sl(src_i),
                        scalar1=0.0,
                        scalar2=None,
                        op0=mybir.AluOpType.is_gt,
                        op1=mybir.AluOpType.add,
                        accum_out=cn(cA),
                    )
                    nc.scalar.activation(
                        out=pl(1),
                        in_=sl(src_t),
                        func=mybir.ActivationFunctionType.Sign,
                        accum_out=cn(cA + 1),
                    )
                    # TP: out = (part_i > 0) * plane_t, accum_out = sum(out)
                    nc.vector.scalar_tensor_tensor(
                        out=pl(2),
                        in0=sl(src_i),
                        scalar=0.0,
                        in1=pl(1),
                        op0=mybir.AluOpType.is_gt,
                        op1=mybir.AluOpType.mult,
                        accum_out=cn(cA + 2),
                    )

            nc.sync.dma_start(out=out, in_=cnts[:])

    nc.compile()
    return nc


def _get_nc():
    global _NC
    if _NC is None:
        _NC = _build_nc()
    return _NC


def _numpy_fallback(inp, tar, lens_index, t):
    """Exact reference semantics in numpy; used only if the inputs deviate
    from the graded configuration (non-arange lens_index or t != 0.5)."""
    Bb, Ss = lens_index.shape
    Ww = inp.shape[1]
    Ll = Ss + Ww - 1
    acc_i = np.zeros((Bb, Ll), np.float32)
    cnt_i = np.zeros((Bb, Ll), np.float32)
    acc_t = np.zeros((Bb, Ll), np.float32)
    cnt_t = np.zeros((Bb, Ll), np.float32)
    for o in range(Ww):
        xi = inp[lens_index, o]
        xt = tar[lens_index, o]
        acc_i[:, o : o + Ss] += xi
        cnt_i[:, o : o + Ss] += (xi != 0)
        acc_t[:, o : o + Ss] += xt
        cnt_t[:, o : o + Ss] += (xt != 0)
    cnt_i[cnt_i <= 0] = 1
    cnt_t[cnt_t <= 0] = 1
    avg_i = (acc_i / cnt_i).astype(np.float64)
    avg_t = acc_t / cnt_t
    raw_i = (1.0 / (1.0 + np.exp(-avg_i)) > t).astype(np.int64)
    raw_t = np.trunc(avg_t).astype(np.int64)
    TP = int(np.sum(raw_i & raw_t))
    TN = int(np.sum((raw_i == 0) & (raw_t == 0)))
    FP = int(np.sum((raw_i == 1) & (raw_t == 0)))
    FN = int(np.sum((raw_i == 0) & (raw_t == 1)))
    return (np.int32(TP), np.int32(TN), np.int32(FP), np.int32(FN))


def kernel(**inputs):
    global LAST_RESULTS
    inp = np.ascontiguousarray(np.asarray(inputs["input"], dtype=np.float32))
    tar = np.ascontiguousarray(np.asarray(inputs["target"], dtype=np.float32))
    lens_index = np.asarray(inputs["lens_index"])
    t = float(np.asarray(inputs["t"]))

    if (
        inp.shape != (N, W)
        or tar.shape != (N, W)
        or lens_index.shape != (B, S)
        or t != 0.5
        or not np.array_equal(
            lens_index.reshape(-1), np.arange(N, dtype=lens_index.dtype)
        )
    ):
        return _numpy_fallback(inp, tar, lens_index, t)

    from concourse.bass_utils import run_bass_kernel_spmd

    nc = _get_nc()
    in_maps = []
    for c in range(NCORES):
        lo = c * SHARD_ROWS
        in_maps.append(
            {"inp": inp[lo : lo + SHARD_ROWS], "tar": tar[lo : lo + SHARD_ROWS]}
        )
    res = run_bass_kernel_spmd(nc, in_maps, core_ids=list(range(NCORES)))
    LAST_RESULTS = res

    CI = CT = TP = 0.0
    for r in res.results:
        o = np.asarray(r["out"], dtype=np.float64)
        CI += o[:, [0, 3, 6, 9]].sum()
        CT += o[:, [1, 4, 7, 10]].sum()
        TP += o[:, [2, 5, 8, 11]].sum()
    CI, CT, TP = int(round(CI)), int(round(CT)), int(round(TP))
    FP = CI - TP
    FN = CT - TP
    TN = B * L - CI - CT + TP
    return (np.int32(TP), np.int32(TN), np.int32(FP), np.int32(FN))


# revision 24
# speedup vs baseline: 1.0072x; 1.0072x over previous
"""Trainium2 Bass kernel for nn_EvalCriterion (segment_reduce confusion counts).

Problem: windows of length W=30 are overlap-added onto a [B, L] grid
(L = S + W - 1), averaged by nonzero-contribution count, thresholded
(sigmoid(avg_i) > t for predictions, trunc(avg_t) for binary labels), and
reduced to four global confusion counts (TP, TN, FP, FN).

Math used by this kernel (valid for the graded configuration):
  * lens_index == arange(N).reshape(B, S)  -> the gather is a plain reshape.
  * t == 0.5  -> sigmoid(acc/cnt) > 0.5  <=>  acc > 0 (cnt is always >= 1),
    so the nonzero-count divisor never needs to be computed.
  * target values are in {0, 1} -> trunc(acc_t/cnt_t) == (acc_t > 0).
  So only the overlap-add sums acc_i, acc_t are needed, then sign tests and
  three global counts: CI = sum(raw_i), CT = sum(raw_t), TP = sum(raw_i*raw_t).
  FP = CI - TP, FN = CT - TP, TN = B*L - CI - CT + TP.

Sharding: data-parallel over B across 8 cores (2 batches per core). Each core
streams its 2x[S, W] blocks of input and target through SBUF once.

Per-core layout: for one batch, partition r holds rows [512r, 512r + 512).
The overlap-add over those rows is a single strided tensor_reduce: with
o' = W-1-o, position q's sum reads offsets 30q + 29o' (all-positive strides,
pairwise distinct), over a tile with 841-element zero guards on both sides.
Positions q in [0, 29) then still need the previous partition's tail
(q in [512, 541)), merged with one partition-shifted SBUF copy + add.
"""

import numpy as np

W = 30
B, S = 16, 65536
N = B * S
L = S + W - 1
NCORES = 8
BPC = B // NCORES          # batches per core = 2
SHARD_ROWS = BPC * S       # rows of input/target per core
P = 128                    # SBUF partitions
RPP = S // P               # rows per partition per batch = 512
LPART = RPP + W - 1        # local acc length = 541
DATA = RPP * W             # data elements per partition = 15360
GUARD = (W - 1) * (W - 1)  # zero guard on each side = 841
TOT = DATA + 2 * GUARD     # tile free size = 17042
K_DVE = 18                 # window offsets summed by the DVE strided reduce
GP_OFFS = W - K_DVE        # window offsets summed by GpSimd shifted adds

_NC = None
LAST_RESULTS = None        # BassKernelResults of the most recent device run


def _build_nc():
    import concourse.bacc as bacc
    import concourse.bass as bass
    import concourse.mybir as mybir
    from concourse.bass import _add_dep_helper
    from concourse.tile import TileContext

    f32 = mybir.dt.float32
    nc = bacc.Bacc(
        "TRN2",
        target_bir_lowering=False,
        debug=False,
        enable_asserts=False,
        num_devices=NCORES,
    )
    inp = nc.dram_tensor("inp", [SHARD_ROWS, W], f32, kind="ExternalInput").ap()
    tar = nc.dram_tensor("tar", [SHARD_ROWS, W], f32, kind="ExternalInput").ap()
    out = nc.dram_tensor("out", [P, 16], f32, kind="ExternalOutput").ap()

    with TileContext(nc) as tc:
        with (
            tc.tile_pool(name="main", bufs=1) as pool,
            tc.tile_pool(name="psum", bufs=1, space="PSUM") as psum,
        ):
            big = [pool.tile([P, TOT], f32, name=f"big{i}", tag=f"big{i}") for i in range(2)]
            parts = [pool.tile([P, LPART], f32, name=f"part{i}", tag=f"part{i}") for i in range(4)]
            accg = [pool.tile([P, LPART], f32, name=f"accg{i}", tag=f"accg{i}") for i in range(4)]
            planes = [pool.tile([P, RPP], f32, name=f"plane{i}", tag=f"plane{i}") for i in range(3)]
            cnts = pool.tile([P, 16], f32, name="cnts", tag="cnts")
            wsh = pool.tile([P, P], f32, name="wsh", tag="wsh")
            ii = pool.tile([P, P], mybir.dt.int32, name="ii", tag="ii")

            # ii[pi, j] = j - pi; wsh[pi, po] = 1 iff po == pi - 1: a matmul
            # with wsh as lhsT shifts partition r+1's row into partition r
            # (row P-1 reads as zero).
            nc.gpsimd.iota(ii[:], pattern=[[1, P]], base=0, channel_multiplier=-1)
            nc.vector.tensor_scalar(
                out=wsh[:], in0=ii[:], scalar1=-1.0, scalar2=None,
                op0=mybir.AluOpType.is_equal,
            )
            nc.gpsimd.memset(cnts[:], 0.0)
            for t in big:
                nc.gpsimd.memset(t[:, 0:GUARD], 0.0)
                nc.gpsimd.memset(t[:, GUARD + DATA : TOT], 0.0)

            srcs = (inp, tar)
            # Stream order: (b0, inp), (b0, tar), (b1, inp), (b1, tar);
            # ping-pong between the two big tiles. Window offsets split:
            # DVE sums o in [GP_OFFS, W) with strided reduces (per DMA
            # chunk), GpSimd sums o in [0, GP_OFFS) with shifted adds into
            # accg; combine deferred one tile and order-pinned after the
            # next tile's reduces so the DVE never head-of-line stalls.
            last_reds = {}
            combines = []
            gp_adds = (
                []
            )  # (k, list of (out_slice, src_ap)) for interleaved emission
            for k in range(2 * BPC):
                b, which = divmod(k, 2)
                t = big[k % 2]
                view = srcs[which][b * S : (b + 1) * S, :].rearrange(
                    "(r m) w -> r (m w)", r=P
                )
                full = t[:]
                NCH = 8 if k == 0 else 4
                CH = RPP // NCH
                qb = [CH * i for i in range(NCH)] + [LPART]
                for ci in range(NCH):
                    nc.sync.dma_start(
                        out=t[:, GUARD + ci * CH * W : GUARD + (ci + 1) * CH * W],
                        in_=view[:, ci * CH * W : (ci + 1) * CH * W],
                    )
                    q0, q1 = qb[ci], qb[ci + 1]
                    red = bass.AP(
                        tensor=full.tensor,
                        offset=full.offset + q0 * W,
                        ap=[list(full.ap[0]), [W, q1 - q0], [W - 1, K_DVE]],
                    )
                    r = nc.vector.tensor_reduce(
                        out=parts[k][:, q0:q1],
                        in_=red,
                        axis=mybir.AxisListType.X,
                        op=mybir.AluOpType.add,
                    )
                    last_reds[k] = r
                nc.gpsimd.memset(accg[k][:], 0.0)
                # last tile: row-halved adds so GpSimd starts at half-arrival
                HH = RPP // 2 if k == 2 * BPC - 1 else RPP
                for h in range(RPP // HH):
                    for o in range(GP_OFFS):
                        srcap = bass.AP(
                            tensor=full.tensor,
                            offset=full.offset + GUARD + h * HH * W + o,
                            ap=[list(full.ap[0]), [W, HH]],
                        )
                        nc.gpsimd.tensor_tensor(
                            out=accg[k][:, h * HH + o : h * HH + o + HH],
                            in0=accg[k][:, h * HH + o : h * HH + o + HH],
                            in1=srcap,
                            op=mybir.AluOpType.add,
                        )
                cmb = nc.vector.tensor_add(
                    out=parts[k][:], in0=parts[k][:], in1=accg[k][:]
                )
                combines.append((k, cmb))

            # pin each combine after the NEXT tile's last reduce: its GpSimd
            # input is only ready around then, and an earlier slot in the DVE
            # stream would head-of-line stall the engine.
            for k, cmb in combines:
                if k + 1 in last_reds:
                    _add_dep_helper(
                        cmb.ins, last_reds[k + 1].ins, sync=False,
                        reason="slot combine after next tile reduces",
                    )

            for b in range(BPC):
                for j in range(2):
                    pt = parts[2 * b + j]
                    ps = psum.tile([P, W - 1], f32, name=f"ps{b}{j}", tag=f"ps{j}")
                    # shift partition r+1's head down to partition r via the
                    # subdiagonal matmul, then add into partition r's tail:
                    # part[r, 512+q] becomes the final value of boundary
                    # position 512*(r+1)+q (row P-1's tail is already final).
                    nc.tensor.matmul(
                        ps[:], wsh[:], pt[:, 0 : W - 1], start=True, stop=True
                    )
                    nc.any.tensor_tensor(
                        out=pt[:, RPP:LPART],
                        in0=pt[:, RPP:LPART],
                        in1=ps[:],
                        op=mybir.AluOpType.add,
                    )

                # Count region A: q in [29, 541) on all partitions (interior +
                # merged boundary), region B: partition 0's head q in [0, 29).
                # Every one of the batch's L positions is counted exactly once.
                # CT and raw_t go through the Scalar engine: acc_t >= 0, so
                # sign(acc_t) == (acc_t > 0) exactly.
                c0 = 6 * b
                for reg, cA in ((0, c0), (1, c0 + 3)):
                    if reg == 0:
                        sl = lambda pt: pt[:, W - 1 : LPART]
                        pl = lambda i: planes[i][:]
                        cn = lambda col: cnts[:, col : col + 1]
                    else:
                        sl = lambda pt: pt[0:1, 0 : W - 1]
                        pl = lambda i: planes[i][0:1, 0 : W - 1]
                        cn = lambda col: cnts[0:1, col : col + 1]
                    nc.vector.tensor_scalar(
                        out=pl(0),
                        in0=sl(parts[2 * b]),
                        scalar1=0.0,
                        scalar2=None,
                        op0=mybir.AluOpType.is_gt,
                        op1=mybir.AluOpType.add,
                        accum_out=cn(cA),
                    )
                    nc.scalar.activation(
                        out=pl(1),
                        in_=sl(parts[2 * b + 1]),
                        func=mybir.ActivationFunctionType.Sign,
                        accum_out=cn(cA + 1),
                    )
                    nc.vector.scalar_tensor_tensor(
                        out=pl(2),
                        in0=sl(parts[2 * b]),
                        scalar=0.0,
                        in1=pl(1),
                        op0=mybir.AluOpType.is_gt,
                        op1=mybir.AluOpType.mult,
                        accum_out=cn(cA + 2),
                    )

            nc.sync.dma_start(out=out, in_=cnts[:])

    nc.compile()
    return nc


def _get_nc():
    global _NC
    if _NC is None:
        _NC = _build_nc()
    return _NC


def _numpy_fallback(inp, tar, lens_index, t):
    """Exact reference semantics in numpy; used only if the inputs deviate
    from the graded configuration (non-arange lens_index or t != 0.5)."""
    Bb, Ss = lens_index.shape
    Ww = inp.shape[1]
    Ll = Ss + Ww - 1
    acc_i = np.zeros((Bb, Ll), np.float32)
    cnt_i = np.zeros((Bb, Ll), np.float32)
    acc_t = np.zeros((Bb, Ll), np.float32)
    cnt_t = np.zeros((Bb, Ll), np.float32)
    for o in range(Ww):
        xi = inp[lens_index, o]
        xt = tar[lens_index, o]
        acc_i[:, o : o + Ss] += xi
        cnt_i[:, o : o + Ss] += (xi != 0)
        acc_t[:, o : o + Ss] += xt
        cnt_t[:, o : o + Ss] += (xt != 0)
    cnt_i[cnt_i <= 0] = 1
    cnt_t[cnt_t <= 0] = 1
    avg_i = (acc_i / cnt_i).astype(np.float64)
    avg_t = acc_t / cnt_t
    raw_i = (1.0 / (1.0 + np.exp(-avg_i)) > t).astype(np.int64)
    raw_t = np.trunc(avg_t).astype(np.int64)
    TP = int(np.sum(raw_i & raw_t))
    TN = int(np.sum((raw_i == 0) & (raw_t == 0)))
    FP = int(np.sum((raw_i == 1) & (raw_t == 0)))
    FN = int(np.sum((raw_i == 0) & (raw_t == 1)))
    return (np.int32(TP), np.int32(TN), np.int32(FP), np.int32(FN))


def kernel(**inputs):
    global LAST_RESULTS
    inp = np.ascontiguousarray(np.asarray(inputs["input"], dtype=np.float32))
    tar = np.ascontiguousarray(np.asarray(inputs["target"], dtype=np.float32))
    lens_index = np.asarray(inputs["lens_index"])
    t = float(np.asarray(inputs["t"]))

    if (
        inp.shape != (N, W)
        or tar.shape != (N, W)
        or lens_index.shape != (B, S)
        or t != 0.5
        or not np.array_equal(
            lens_index.reshape(-1), np.arange(N, dtype=lens_index.dtype)
        )
    ):
        return _numpy_fallback(inp, tar, lens_index, t)

    from concourse.bass_utils import run_bass_kernel_spmd

    nc = _get_nc()
    in_maps = []
    for c in range(NCORES):
        lo = c * SHARD_ROWS
        in_maps.append(
            {"inp": inp[lo : lo + SHARD_ROWS], "tar": tar[lo : lo + SHARD_ROWS]}
        )
    res = run_bass_kernel_spmd(nc, in_maps, core_ids=list(range(NCORES)))
    LAST_RESULTS = res

    CI = CT = TP = 0.0
    for r in res.results:
        o = np.asarray(r["out"], dtype=np.float64)
        CI += o[:, [0, 3, 6, 9]].sum()
        CT += o[:, [1, 4, 7, 10]].sum()
        TP += o[:, [2, 5, 8, 11]].sum()
    CI, CT, TP = int(round(CI)), int(round(CT)), int(round(TP))
    FP = CI - TP
    FN = CT - TP
    TN = B * L - CI - CT + TP
    return (np.int32(TP), np.int32(TN), np.int32(FP), np.int32(FN))


# revision 25
# speedup vs baseline: 1.0183x; 1.0110x over previous
"""Trainium2 Bass kernel for nn_EvalCriterion (segment_reduce confusion counts).

Problem: windows of length W=30 are overlap-added onto a [B, L] grid
(L = S + W - 1), averaged by nonzero-contribution count, thresholded
(sigmoid(avg_i) > t for predictions, trunc(avg_t) for binary labels), and
reduced to four global confusion counts (TP, TN, FP, FN).

Math used by this kernel (valid for the graded configuration):
  * lens_index == arange(N).reshape(B, S)  -> the gather is a plain reshape.
  * t == 0.5  -> sigmoid(acc/cnt) > 0.5  <=>  acc > 0 (cnt is always >= 1),
    so the nonzero-count divisor never needs to be computed.
  * target values are in {0, 1} -> trunc(acc_t/cnt_t) == (acc_t > 0).
  So only the overlap-add sums acc_i, acc_t are needed, then sign tests and
  three global counts: CI = sum(raw_i), CT = sum(raw_t), TP = sum(raw_i*raw_t).
  FP = CI - TP, FN = CT - TP, TN = B*L - CI - CT + TP.

Sharding: data-parallel over B across 8 cores (2 batches per core). Each core
streams its 2x[S, W] blocks of input and target through SBUF once.

Per-core layout: for one batch, partition r holds rows [512r, 512r + 512).
The overlap-add over those rows is a single strided tensor_reduce: with
o' = W-1-o, position q's sum reads offsets 30q + 29o' (all-positive strides,
pairwise distinct), over a tile with 841-element zero guards on both sides.
Positions q in [0, 29) then still need the previous partition's tail
(q in [512, 541)), merged with one partition-shifted SBUF copy + add.
"""

import numpy as np

W = 30
B, S = 16, 65536
N = B * S
L = S + W - 1
NCORES = 8
BPC = B // NCORES          # batches per core = 2
SHARD_ROWS = BPC * S       # rows of input/target per core
P = 128                    # SBUF partitions
RPP = S // P               # rows per partition per batch = 512
LPART = RPP + W - 1        # local acc length = 541
DATA = RPP * W             # data elements per partition = 15360
GUARD = (W - 1) * (W - 1)  # zero guard on each side = 841
TOT = DATA + 2 * GUARD     # tile free size = 17042
K_DVE = 18                 # window offsets summed by the DVE strided reduce
GP_OFFS = W - K_DVE        # window offsets summed by GpSimd shifted adds

_NC = None
LAST_RESULTS = None        # BassKernelResults of the most recent device run


def _build_nc():
    import concourse.bacc as bacc
    import concourse.bass as bass
    import concourse.mybir as mybir
    from concourse.bass import _add_dep_helper
    from concourse.tile import TileContext

    f32 = mybir.dt.float32
    nc = bacc.Bacc(
        "TRN2",
        target_bir_lowering=False,
        debug=False,
        enable_asserts=False,
        num_devices=NCORES,
    )
    inp = nc.dram_tensor("inp", [SHARD_ROWS, W], f32, kind="ExternalInput").ap()
    tar = nc.dram_tensor("tar", [SHARD_ROWS, W], f32, kind="ExternalInput").ap()
    out = nc.dram_tensor("out", [P, 16], f32, kind="ExternalOutput").ap()

    with TileContext(nc) as tc:
        with (
            tc.tile_pool(name="main", bufs=1) as pool,
            tc.tile_pool(name="psum", bufs=1, space="PSUM") as psum,
        ):
            big = [pool.tile([P, TOT], f32, name=f"big{i}", tag=f"big{i}") for i in range(2)]
            parts = [pool.tile([P, LPART], f32, name=f"part{i}", tag=f"part{i}") for i in range(4)]
            accg = [pool.tile([P, LPART], f32, name=f"accg{i}", tag=f"accg{i}") for i in range(4)]
            planes = [pool.tile([P, RPP], f32, name=f"plane{i}", tag=f"plane{i}") for i in range(3)]
            cnts = pool.tile([P, 16], f32, name="cnts", tag="cnts")
            wsh = pool.tile([P, P], f32, name="wsh", tag="wsh")
            ii = pool.tile([P, P], mybir.dt.int32, name="ii", tag="ii")

            # ii[pi, j] = j - pi; wsh[pi, po] = 1 iff po == pi - 1: a matmul
            # with wsh as lhsT shifts partition r+1's row into partition r
            # (row P-1 reads as zero).
            nc.gpsimd.iota(ii[:], pattern=[[1, P]], base=0, channel_multiplier=-1)
            nc.vector.tensor_scalar(
                out=wsh[:], in0=ii[:], scalar1=-1.0, scalar2=None,
                op0=mybir.AluOpType.is_equal,
            )
            nc.gpsimd.memset(cnts[:], 0.0)
            for t in big:
                nc.gpsimd.memset(t[:, 0:GUARD], 0.0)
                nc.gpsimd.memset(t[:, GUARD + DATA : TOT], 0.0)

            srcs = (inp, tar)
            # Stream order: (b0, inp), (b0, tar), (b1, inp), (b1, tar);
            # ping-pong between the two big tiles. Window offsets split:
            # DVE sums o in [GP_OFFS, W) with strided reduces (per DMA
            # chunk), GpSimd sums o in [0, GP_OFFS) with shifted adds into
            # accg; combine deferred one tile and order-pinned after the
            # next tile's reduces so the DVE never head-of-line stalls.
            last_reds = {}
            combines = []
            gp_adds = (
                []
            )  # (k, list of (out_slice, src_ap)) for interleaved emission
            for k in range(2 * BPC):
                b, which = divmod(k, 2)
                t = big[k % 2]
                view = srcs[which][b * S : (b + 1) * S, :].rearrange(
                    "(r m) w -> r (m w)", r=P
                )
                full = t[:]
                NCH = 8 if k == 0 else 4
                CH = RPP // NCH
                qb = [CH * i for i in range(NCH)] + [LPART]
                for ci in range(NCH):
                    nc.sync.dma_start(
                        out=t[:, GUARD + ci * CH * W : GUARD + (ci + 1) * CH * W],
                        in_=view[:, ci * CH * W : (ci + 1) * CH * W],
                    )
                    q0, q1 = qb[ci], qb[ci + 1]
                    red = bass.AP(
                        tensor=full.tensor,
                        offset=full.offset + q0 * W,
                        ap=[list(full.ap[0]), [W, q1 - q0], [W - 1, K_DVE]],
                    )
                    r = nc.vector.tensor_reduce(
                        out=parts[k][:, q0:q1],
                        in_=red,
                        axis=mybir.AxisListType.X,
                        op=mybir.AluOpType.add,
                    )
                    last_reds[k] = r
                nc.gpsimd.memset(accg[k][:], 0.0)
                # last tile: row-halved adds so GpSimd starts at half-arrival
                HH = RPP
                for h in range(RPP // HH):
                    for o in range(GP_OFFS):
                        srcap = bass.AP(
                            tensor=full.tensor,
                            offset=full.offset + GUARD + h * HH * W + o,
                            ap=[list(full.ap[0]), [W, HH]],
                        )
                        nc.gpsimd.tensor_tensor(
                            out=accg[k][:, h * HH + o : h * HH + o + HH],
                            in0=accg[k][:, h * HH + o : h * HH + o + HH],
                            in1=srcap,
                            op=mybir.AluOpType.add,
                        )
                cmb = nc.vector.tensor_add(
                    out=parts[k][:], in0=parts[k][:], in1=accg[k][:]
                )
                combines.append((k, cmb))

            # pin each combine after the NEXT tile's last reduce: its GpSimd
            # input is only ready around then, and an earlier slot in the DVE
            # stream would head-of-line stall the engine.
            for k, cmb in combines:
                if k + 1 in last_reds:
                    _add_dep_helper(
                        cmb.ins, last_reds[k + 1].ins, sync=False,
                        reason="slot combine after next tile reduces",
                    )

            for b in range(BPC):
                for j in range(2):
                    pt = parts[2 * b + j]
                    ps = psum.tile([P, W - 1], f32, name=f"ps{b}{j}", tag=f"ps{j}")
                    # shift partition r+1's head down to partition r via the
                    # subdiagonal matmul, then add into partition r's tail:
                    # part[r, 512+q] becomes the final value of boundary
                    # position 512*(r+1)+q (row P-1's tail is already final).
                    nc.tensor.matmul(
                        ps[:], wsh[:], pt[:, 0 : W - 1], start=True, stop=True
                    )
                    nc.any.tensor_tensor(
                        out=pt[:, RPP:LPART],
                        in0=pt[:, RPP:LPART],
                        in1=ps[:],
                        op=mybir.AluOpType.add,
                    )

                # Count region A: q in [29, 541) on all partitions (interior +
                # merged boundary), region B: partition 0's head q in [0, 29).
                # Every one of the batch's L positions is counted exactly once.
                # CT and raw_t go through the Scalar engine: acc_t >= 0, so
                # sign(acc_t) == (acc_t > 0) exactly.
                c0 = 6 * b
                for reg, cA in ((0, c0), (1, c0 + 3)):
                    if reg == 0:
                        sl = lambda pt: pt[:, W - 1 : LPART]
                        pl = lambda i: planes[i][:]
                        cn = lambda col: cnts[:, col : col + 1]
                    else:
                        sl = lambda pt: pt[0:1, 0 : W - 1]
                        pl = lambda i: planes[i][0:1, 0 : W - 1]
                        cn = lambda col: cnts[0:1, col : col + 1]
                    nc.vector.tensor_scalar(
                        out=pl(0),
                        in0=sl(parts[2 * b]),
                        scalar1=0.0,
                        scalar2=None,
                        op0=mybir.AluOpType.is_gt,
                        op1=mybir.AluOpType.add,
                        accum_out=cn(cA),
                    )
                    nc.scalar.activation(
                        out=pl(1),
                        in_=sl(parts[2 * b + 1]),
                        func=mybir.ActivationFunctionType.Sign,
                        accum_out=cn(cA + 1),
                    )
                    nc.vector.scalar_tensor_tensor(
                        out=pl(2),
                        in0=sl(parts[2 * b]),
                        scalar=0.0,
                        in1=pl(1),
                        op0=mybir.AluOpType.is_gt,
                        op1=mybir.AluOpType.mult,
                        accum_out=cn(cA + 2),
                    )

            nc.sync.dma_start(out=out, in_=cnts[:])

    nc.compile()
    return nc


def _get_nc():
    global _NC
    if _NC is None:
        _NC = _build_nc()
    return _NC


def _numpy_fallback(inp, tar, lens_index, t):
    """Exact reference semantics in numpy; used only if the inputs deviate
    from the graded configuration (non-arange lens_index or t != 0.5)."""
    Bb, Ss = lens_index.shape
    Ww = inp.shape[1]
    Ll = Ss + Ww - 1
    acc_i = np.zeros((Bb, Ll), np.float32)
    cnt_i = np.zeros((Bb, Ll), np.float32)
    acc_t = np.zeros((Bb, Ll), np.float32)
    cnt_t = np.zeros((Bb, Ll), np.float32)
    for o in range(Ww):
        xi = inp[lens_index, o]
        xt = tar[lens_index, o]
        acc_i[:, o : o + Ss] += xi
        cnt_i[:, o : o + Ss] += (xi != 0)
        acc_t[:, o : o + Ss] += xt
        cnt_t[:, o : o + Ss] += (xt != 0)
    cnt_i[cnt_i <= 0] = 1
    cnt_t[cnt_t <= 0] = 1
    avg_i = (acc_i / cnt_i).astype(np.float64)
    avg_t = acc_t / cnt_t
    raw_i = (1.0 / (1.0 + np.exp(-avg_i)) > t).astype(np.int64)
    raw_t = np.trunc(avg_t).astype(np.int64)
    TP = int(np.sum(raw_i & raw_t))
    TN = int(np.sum((raw_i == 0) & (raw_t == 0)))
    FP = int(np.sum((raw_i == 1) & (raw_t == 0)))
    FN = int(np.sum((raw_i == 0) & (raw_t == 1)))
    return (np.int32(TP), np.int32(TN), np.int32(FP), np.int32(FN))


def kernel(**inputs):
    global LAST_RESULTS
    inp = np.ascontiguousarray(np.asarray(inputs["input"], dtype=np.float32))
    tar = np.ascontiguousarray(np.asarray(inputs["target"], dtype=np.float32))
    lens_index = np.asarray(inputs["lens_index"])
    t = float(np.asarray(inputs["t"]))

    if (
        inp.shape != (N, W)
        or tar.shape != (N, W)
        or lens_index.shape != (B, S)
        or t != 0.5
        or not np.array_equal(
            lens_index.reshape(-1), np.arange(N, dtype=lens_index.dtype)
        )
    ):
        return _numpy_fallback(inp, tar, lens_index, t)

    from concourse.bass_utils import run_bass_kernel_spmd

    nc = _get_nc()
    in_maps = []
    for c in range(NCORES):
        lo = c * SHARD_ROWS
        in_maps.append(
            {"inp": inp[lo : lo + SHARD_ROWS], "tar": tar[lo : lo + SHARD_ROWS]}
        )
    res = run_bass_kernel_spmd(nc, in_maps, core_ids=list(range(NCORES)))
    LAST_RESULTS = res

    CI = CT = TP = 0.0
    for r in res.results:
        o = np.asarray(r["out"], dtype=np.float64)
        CI += o[:, [0, 3, 6, 9]].sum()
        CT += o[:, [1, 4, 7, 10]].sum()
        TP += o[:, [2, 5, 8, 11]].sum()
    CI, CT, TP = int(round(CI)), int(round(CT)), int(round(TP))
    FP = CI - TP
    FN = CT - TP
    TN = B * L - CI - CT + TP
    return (np.int32(TP), np.int32(TN), np.int32(FP), np.int32(FN))


# revision 26
# speedup vs baseline: 1.1084x; 1.0884x over previous
"""Trainium2 Bass kernel for nn_EvalCriterion (segment_reduce confusion counts).

Problem: windows of length W=30 are overlap-added onto a [B, L] grid
(L = S + W - 1), averaged by nonzero-contribution count, thresholded
(sigmoid(avg_i) > t for predictions, trunc(avg_t) for binary labels), and
reduced to four global confusion counts (TP, TN, FP, FN).

Math used by this kernel (valid for the graded configuration):
  * lens_index == arange(N).reshape(B, S)  -> the gather is a plain reshape.
  * t == 0.5  -> sigmoid(acc/cnt) > 0.5  <=>  acc > 0 (cnt is always >= 1),
    so the nonzero-count divisor never needs to be computed.
  * target values are in {0, 1} -> trunc(acc_t/cnt_t) == (acc_t > 0).
  So only the overlap-add sums acc_i, acc_t are needed, then sign tests and
  three global counts: CI = sum(raw_i), CT = sum(raw_t), TP = sum(raw_i*raw_t).
  FP = CI - TP, FN = CT - TP, TN = B*L - CI - CT + TP.

Sharding: data-parallel over B across 8 cores (2 batches per core). Each core
streams its 2x[S, W] blocks of input and target through SBUF once.

Per-core layout: for one batch, partition r holds rows [512r, 512r + 512).
The overlap-add over those rows is a single strided tensor_reduce: with
o' = W-1-o, position q's sum reads offsets 30q + 29o' (all-positive strides,
pairwise distinct), over a tile with 841-element zero guards on both sides.
Positions q in [0, 29) then still need the previous partition's tail
(q in [512, 541)), merged with one partition-shifted SBUF copy + add.
"""

import numpy as np

W = 30
B, S = 16, 65536
N = B * S
L = S + W - 1
NCORES = 8
BPC = B // NCORES          # batches per core = 2
SHARD_ROWS = BPC * S       # rows of input/target per core
P = 128                    # SBUF partitions
RPP = S // P               # rows per partition per batch = 512
LPART = RPP + W - 1        # local acc length = 541
DATA = RPP * W             # data elements per partition = 15360
GUARD = (W - 1) * (W - 1)  # zero guard on each side = 841
TOT = DATA + 2 * GUARD     # tile free size = 17042
K_DVE = 18                 # window offsets summed by the DVE strided reduce
GP_OFFS = W - K_DVE        # window offsets summed by GpSimd shifted adds

_NC = None
LAST_RESULTS = None        # BassKernelResults of the most recent device run


def _build_nc():
    import concourse.bacc as bacc
    import concourse.bass as bass
    import concourse.mybir as mybir
    from concourse.bass import _add_dep_helper
    from concourse.tile import TileContext

    f32 = mybir.dt.float32
    nc = bacc.Bacc(
        "TRN2",
        target_bir_lowering=False,
        debug=False,
        enable_asserts=False,
        num_devices=NCORES,
    )
    inp = nc.dram_tensor("inp", [SHARD_ROWS, W], f32, kind="ExternalInput").ap()
    tar = nc.dram_tensor("tar", [SHARD_ROWS, W], f32, kind="ExternalInput").ap()
    out = nc.dram_tensor("out", [P, 16], f32, kind="ExternalOutput").ap()

    with TileContext(nc) as tc:
        with (
            tc.tile_pool(name="main", bufs=1) as pool,
            tc.tile_pool(name="psum", bufs=1, space="PSUM") as psum,
        ):
            big = [pool.tile([P, TOT], f32, name=f"big{i}", tag=f"big{i}") for i in range(2)]
            parts = [pool.tile([P, LPART], f32, name=f"part{i}", tag=f"part{i}") for i in range(4)]
            accg = [pool.tile([P, LPART], f32, name=f"accg{i}", tag=f"accg{i}") for i in range(4)]
            planes = [pool.tile([P, RPP], f32, name=f"plane{i}", tag=f"plane{i}") for i in range(3)]
            cnts = pool.tile([P, 16], f32, name="cnts", tag="cnts")
            wsh = pool.tile([P, P], f32, name="wsh", tag="wsh")
            ii = pool.tile([P, P], mybir.dt.int32, name="ii", tag="ii")

            # ii[pi, j] = j - pi; wsh[pi, po] = 1 iff po == pi - 1: a matmul
            # with wsh as lhsT shifts partition r+1's row into partition r
            # (row P-1 reads as zero).
            nc.gpsimd.iota(ii[:], pattern=[[1, P]], base=0, channel_multiplier=-1)
            nc.vector.tensor_scalar(
                out=wsh[:], in0=ii[:], scalar1=-1.0, scalar2=None,
                op0=mybir.AluOpType.is_equal,
            )
            nc.gpsimd.memset(cnts[:], 0.0)
            for t in big:
                nc.gpsimd.memset(t[:, 0:GUARD], 0.0)
                nc.gpsimd.memset(t[:, GUARD + DATA : TOT], 0.0)

            srcs = (inp, tar)
            # Stream order: (b0, inp), (b0, tar), (b1, inp), (b1, tar);
            # ping-pong between the two big tiles. Window offsets split:
            # DVE sums o in [GP_OFFS, W) with strided reduces (per DMA
            # chunk), GpSimd sums o in [0, GP_OFFS) with shifted adds into
            # accg; combine deferred one tile and order-pinned after the
            # next tile's reduces so the DVE never head-of-line stalls.
            last_reds = {}
            combines = []
            gp_adds = (
                []
            )  # (k, list of (out_slice, src_ap)) for interleaved emission
            for k in range(2 * BPC):
                b, which = divmod(k, 2)
                t = big[k % 2]
                view = srcs[which][b * S : (b + 1) * S, :].rearrange(
                    "(r m) w -> r (m w)", r=P
                )
                full = t[:]
                NCH = 8 if k == 0 else 4
                CH = RPP // NCH
                qb = [CH * i for i in range(NCH)] + [LPART]
                for ci in range(NCH):
                    nc.sync.dma_start(
                        out=t[:, GUARD + ci * CH * W : GUARD + (ci + 1) * CH * W],
                        in_=view[:, ci * CH * W : (ci + 1) * CH * W],
                    )
                    q0, q1 = qb[ci], qb[ci + 1]
                    red = bass.AP(
                        tensor=full.tensor,
                        offset=full.offset + q0 * W,
                        ap=[list(full.ap[0]), [W, q1 - q0], [W - 1, K_DVE]],
                    )
                    r = nc.vector.tensor_reduce(
                        out=parts[k][:, q0:q1],
                        in_=red,
                        axis=mybir.AxisListType.X,
                        op=mybir.AluOpType.add,
                    )
                    last_reds[k] = r
                nc.gpsimd.memset(accg[k][:], 0.0)
                # last tile: row-halved adds so GpSimd starts at half-arrival
                HH = RPP
                for h in range(RPP // HH):
                    for o in range(GP_OFFS):
                        srcap = bass.AP(
                            tensor=full.tensor,
                            offset=full.offset + GUARD + h * HH * W + o,
                            ap=[list(full.ap[0]), [W, HH]],
                        )
                        nc.gpsimd.tensor_tensor(
                            out=accg[k][:, h * HH + o : h * HH + o + HH],
                            in0=accg[k][:, h * HH + o : h * HH + o + HH],
                            in1=srcap,
                            op=mybir.AluOpType.add,
                        )
                cmb = nc.any.tensor_tensor(
                    out=parts[k][:], in0=parts[k][:], in1=accg[k][:],
                    op=mybir.AluOpType.add,
                )
                combines.append((k, cmb))

            # pin each combine after the NEXT tile's last reduce: its GpSimd
            # input is only ready around then, and an earlier slot in the DVE
            # stream would head-of-line stall the engine.


            for b in range(BPC):
                for j in range(2):
                    pt = parts[2 * b + j]
                    ps = psum.tile([P, W - 1], f32, name=f"ps{b}{j}", tag=f"ps{j}")
                    # shift partition r+1's head down to partition r via the
                    # subdiagonal matmul, then add into partition r's tail:
                    # part[r, 512+q] becomes the final value of boundary
                    # position 512*(r+1)+q (row P-1's tail is already final).
                    nc.tensor.matmul(
                        ps[:], wsh[:], pt[:, 0 : W - 1], start=True, stop=True
                    )
                    nc.any.tensor_tensor(
                        out=pt[:, RPP:LPART],
                        in0=pt[:, RPP:LPART],
                        in1=ps[:],
                        op=mybir.AluOpType.add,
                    )

                # Count region A: q in [29, 541) on all partitions (interior +
                # merged boundary), region B: partition 0's head q in [0, 29).
                # Every one of the batch's L positions is counted exactly once.
                # CT and raw_t go through the Scalar engine: acc_t >= 0, so
                # sign(acc_t) == (acc_t > 0) exactly.
                c0 = 6 * b
                for reg, cA in ((0, c0), (1, c0 + 3)):
                    if reg == 0:
                        sl = lambda pt: pt[:, W - 1 : LPART]
                        pl = lambda i: planes[i][:]
                        cn = lambda col: cnts[:, col : col + 1]
                    else:
                        sl = lambda pt: pt[0:1, 0 : W - 1]
                        pl = lambda i: planes[i][0:1, 0 : W - 1]
                        cn = lambda col: cnts[0:1, col : col + 1]
                    nc.vector.tensor_scalar(
                        out=pl(0),
                        in0=sl(parts[2 * b]),
                        scalar1=0.0,
                        scalar2=None,
                        op0=mybir.AluOpType.is_gt,
                        op1=mybir.AluOpType.add,
                        accum_out=cn(cA),
                    )
                    nc.scalar.activation(
                        out=pl(1),
                        in_=sl(parts[2 * b + 1]),
                        func=mybir.ActivationFunctionType.Sign,
                        accum_out=cn(cA + 1),
                    )
                    nc.vector.scalar_tensor_tensor(
                        out=pl(2),
                        in0=sl(parts[2 * b]),
                        scalar=0.0,
                        in1=pl(1),
                        op0=mybir.AluOpType.is_gt,
                        op1=mybir.AluOpType.mult,
                        accum_out=cn(cA + 2),
                    )

            nc.sync.dma_start(out=out, in_=cnts[:])

    nc.compile()
    return nc


def _get_nc():
    global _NC
    if _NC is None:
        _NC = _build_nc()
    return _NC


def _numpy_fallback(inp, tar, lens_index, t):
    """Exact reference semantics in numpy; used only if the inputs deviate
    from the graded configuration (non-arange lens_index or t != 0.5)."""
    Bb, Ss = lens_index.shape
    Ww = inp.shape[1]
    Ll = Ss + Ww - 1
    acc_i = np.zeros((Bb, Ll), np.float32)
    cnt_i = np.zeros((Bb, Ll), np.float32)
    acc_t = np.zeros((Bb, Ll), np.float32)
    cnt_t = np.zeros((Bb, Ll), np.float32)
    for o in range(Ww):
        xi = inp[lens_index, o]
        xt = tar[lens_index, o]
        acc_i[:, o : o + Ss] += xi
        cnt_i[:, o : o + Ss] += (xi != 0)
        acc_t[:, o : o + Ss] += xt
        cnt_t[:, o : o + Ss] += (xt != 0)
    cnt_i[cnt_i <= 0] = 1
    cnt_t[cnt_t <= 0] = 1
    avg_i = (acc_i / cnt_i).astype(np.float64)
    avg_t = acc_t / cnt_t
    raw_i = (1.0 / (1.0 + np.exp(-avg_i)) > t).astype(np.int64)
    raw_t = np.trunc(avg_t).astype(np.int64)
    TP = int(np.sum(raw_i & raw_t))
    TN = int(np.sum((raw_i == 0) & (raw_t == 0)))
    FP = int(np.sum((raw_i == 1) & (raw_t == 0)))
    FN = int(np.sum((raw_i == 0) & (raw_t == 1)))
    return (np.int32(TP), np.int32(TN), np.int32(FP), np.int32(FN))


def kernel(**inputs):
    global LAST_RESULTS
    inp = np.ascontiguousarray(np.asarray(inputs["input"], dtype=np.float32))
    tar = np.ascontiguousarray(np.asarray(inputs["target"], dtype=np.float32))
    lens_index = np.asarray(inputs["lens_index"])
    t = float(np.asarray(inputs["t"]))

    if (
        inp.shape != (N, W)
        or tar.shape != (N, W)
        or lens_index.shape != (B, S)
        or t != 0.5
        or not np.array_equal(
            lens_index.reshape(-1), np.arange(N, dtype=lens_index.dtype)
        )
    ):
        return _numpy_fallback(inp, tar, lens_index, t)

    from concourse.bass_utils import run_bass_kernel_spmd

    nc = _get_nc()
    in_maps = []
    for c in range(NCORES):
        lo = c * SHARD_ROWS
        in_maps.append(
            {"inp": inp[lo : lo + SHARD_ROWS], "tar": tar[lo : lo + SHARD_ROWS]}
        )
    res = run_bass_kernel_spmd(nc, in_maps, core_ids=list(range(NCORES)))
    LAST_RESULTS = res

    CI = CT = TP = 0.0
    for r in res.results:
        o = np.asarray(r["out"], dtype=np.float64)
        CI += o[:, [0, 3, 6, 9]].sum()
        CT += o[:, [1, 4, 7, 10]].sum()
        TP += o[:, [2, 5, 8, 11]].sum()
    CI, CT, TP = int(round(CI)), int(round(CT)), int(round(TP))
    FP = CI - TP
    FN = CT - TP
    TN = B * L - CI - CT + TP
    return (np.int32(TP), np.int32(TN), np.int32(FP), np.int32(FN))
